# revision 52
# baseline (speedup 1.0000x reference)
"""Causal self-attention on 8 TRN2 NeuronCores.

Sharding: pure data-parallel on batch (B=8 -> one batch element per core,
no collectives). Each core computes its full [T, C] output slice.

Per-core dataflow (all matmuls bf16 with fp32 PSUM accumulation):
  xT [C,T] (host-pretransposed) --+--> qkT = (w_qk^T @ xT) + b_qk  [2C, T]
                                  +--> V   = (xT^T @ w_v) + b_v    [T, C] (padded
                                       with a ones column per head: [.., 65])
  per head-pair u, per q-chunk (512):
    S^T[k,q] = k_h . q_h           (lhsT = kT_h slice, rhs = qT_h slice; the
                                    A/B head halves run concurrently in
                                    disjoint PE row groups)
    E = exp(S^T)                   (ACT, PSUM->SBUF, bf16 out; q pre-scaled
                                    by 1/8 on host so no separate scale op)
    E *= causal mask               (single strided-AP multiply covering only
                                    the 128-col diagonal blocks, on GpSimd)
    O'[d,q], s[q] = [v_h | 1]^T @ E  (augmented-ones row gives softmax sums;
                                      A and B go to slots of one PSUM tile)
    r = 1/s (DVE reciprocal_approx_fast, f32)
    R = partition_broadcast(r)     (GpSimd, SBUF->SBUF)
    Y^T block = O' * R             (DVE)
  Z = (Y^T)^T @ w_proj + b_proj    -> DMA PSUM -> DRAM out

Scheduling: the qkT GEMM is split per head-pair unit and its chains are
interleaved into the attention pipeline as PE filler, so the tensor engine
streams continuously (keeps the HAM clock-gate at 8/8) while exp/recip/
normalize latencies hide underneath. Input DMAs are split so the first
matmul can start ~2us in. Biases enter as K=1 rank-1 accumulate matmuls.
"""

import os
import sys
from contextlib import ExitStack

import numpy as np

try:
    import ml_dtypes
except ImportError:  # pragma: no cover
    sys.path.insert(0, "/opt/trn_rl_repo")
    import ml_dtypes

BF16 = ml_dtypes.bfloat16

B, T, C = 8, 1024, 1024
H, HD = 16, 64
N_CORES = 8

# Toggled by test harness to capture a hardware profile.
TRACE = False
LAST_EXEC_NS = None
LAST_RESULTS = None

_NC_CACHE = {}


def _build_nc(Tp, Cp, Hp, HDp, reps=1, with_bias_v=True, with_bias_p=True):
    import concourse.bass as bass
    import concourse.tile as tile
    from concourse import bacc, mybir

    bf = mybir.dt.bfloat16
    f32 = mybir.dt.float32
    AF = mybir.ActivationFunctionType

    P = 128
    CT = Cp // P            # c-tiles (contraction tiles)
    TT = Tp // P            # t-tiles
    QC = min(512, Tp)       # q-chunk width (free dim per matmul)
    NQ = Tp // QC           # q-chunks
    TCH = min(512, Tp)      # t-chunk width for qkT rhs
    TJ = Tp // TCH
    DIAG = QC // P          # diagonal k-tiles per q-chunk
    M2C = 2 * Cp // P       # qk m-chunks
    VJ = Cp // QC           # v/proj column chunks
    NU = Hp // 2            # head-pair units
    NIT = NU * NQ           # attention iterations (unit, q-chunk)

    nc = bacc.Bacc("TRN2", target_bir_lowering=False, debug=False)

    xT_d = nc.declare_dram_parameter("xT", [Cp, Tp], bf, isOutput=False)
    wqk_d = nc.declare_dram_parameter("w_qk", [Cp, 2 * Cp], bf, isOutput=False)
    wv_d = nc.declare_dram_parameter("w_v", [Cp, Cp], bf, isOutput=False)
    wp_d = nc.declare_dram_parameter("w_proj", [Cp, Cp], bf, isOutput=False)
    bqk_d = nc.declare_dram_parameter("b_qk", [M2C, P], f32, isOutput=False)
    bv_d = nc.declare_dram_parameter("b_v", [1, Cp], bf, isOutput=False)
    bp_d = nc.declare_dram_parameter("b_proj", [1, Cp], bf, isOutput=False)
    # lower-tri [k<=q] 128x128 block, replicated DIAG times along the middle
    # axis to match the strided diagonal-block AP of the E tiles
    mask_d = nc.declare_dram_parameter("masks", [P, DIAG, P], bf, isOutput=False)
    out_d = nc.declare_dram_parameter("out", [Tp, Cp], f32, isOutput=True)

    with tile.TileContext(nc) as tc, ExitStack() as ctx:
        consts = ctx.enter_context(tc.tile_pool(name="consts", bufs=1))
        # E tiles are flat [P, cols]; qj=0 needs 2560 cols, qj=1 needs 4608
        epool0 = ctx.enter_context(tc.tile_pool(name="epool0", bufs=2))
        epool1 = ctx.enter_context(tc.tile_pool(name="epool1", bufs=2))
        rpool2 = ctx.enter_context(tc.tile_pool(name="rpool2", bufs=2))
        zpool = ctx.enter_context(tc.tile_pool(name="zpool", bufs=2))
        # qkT head-pair tiles rotate through 4 slots (live qk-emit -> S use)
        qkpool = ctx.enter_context(tc.tile_pool(name="qkpool", bufs=4))
        # S-pair tiles and the combined AV accumulator share one 3-slot pool
        # (each [P, 2, QC] f32 = 2 banks -> 6 banks) + 2 banks for GEMM chains
        psatt = ctx.enter_context(tc.tile_pool(name="psatt", bufs=3, space="PSUM"))
        psmm = ctx.enter_context(tc.tile_pool(name="psmm", bufs=2, space="PSUM"))

        # ---- persistent SBUF buffers ----
        xT = consts.tile([P, CT, Tp], bf)
        wqk = consts.tile([P, CT, 2 * Cp], bf)
        wv = consts.tile([P, CT, Cp], bf)
        wp = consts.tile([P, CT, Cp], bf)
        # per head: [ones | 63 pad | v0..v63] (128 cols) -> softmax-sum row
        # lands at PSUM partition 0 (where the custom-DVE reciprocal reads)
        # and O' at partitions 64:128 (legal 64-partition base)
        VW = 2 * HDp
        Vp = consts.tile([P, TT, Hp, VW], bf)
        YT = consts.tile([P, CT, Tp], bf)
        bqk = consts.tile([P, M2C], f32)
        bv = bp = None
        if with_bias_v:
            bv = consts.tile([1, Cp], bf)
        if with_bias_p:
            bp = consts.tile([1, Cp], bf)
        ones = None
        if with_bias_v or with_bias_p:
            ones = consts.tile([1, max(QC, P)], bf)
        masks = consts.tile([P, DIAG, P], bf)

        # ---- input DMAs, priority-ordered and split for early PE start ----
        # wqk column pairs: unit u needs cols [128u,128u+128) and Cp+[...]
        def _dma_wqk(lo_u, hi_u):
            w = (hi_u - lo_u) * P
            for base in (0, Cp):
                nc.sync.dma_start(
                    wqk[:, :, base + lo_u * P: base + lo_u * P + w],
                    wqk_d[:, base + lo_u * P: base + lo_u * P + w].rearrange(
                        "(ct p) n -> p ct n", p=P
                    ),
                )

        _dma_wqk(0, 2)  # units 0-1 first
        for ct in range(CT):
            nc.sync.dma_start(
                xT[:, ct, :], xT_d[ct * P: (ct + 1) * P, :]
            )
        nc.sync.dma_start(bqk[:], bqk_d.rearrange("m p -> p m"))
        nc.sync.dma_start(wv[:], wv_d.rearrange("(ct p) n -> p ct n", p=P))
        nc.sync.dma_start(masks[:], mask_d[:])
        _dma_wqk(2, NU)  # remaining units
        nc.sync.dma_start(wp[:], wp_d.rearrange("(ct p) n -> p ct n", p=P))
        if with_bias_v:
            nc.sync.dma_start(bv[:], bv_d[:])
        if with_bias_p:
            nc.sync.dma_start(bp[:], bp_d[:])
        if ones is not None:
            nc.gpsimd.memset(ones[:], 1.0)
        nc.gpsimd.memset(Vp[:], 1.0)  # ones column survives; rest overwritten

        qkt_tiles = {}

        def _emit_qk_chain(u, half, tj):
            # one qkT chain: [P, TCH] psum accumulated over CT, then bias+copy
            # into slot `half` (0=q, 1=k) of unit u's rotating qkT tile
            if u not in qkt_tiles:
                qkt_tiles[u] = qkpool.tile(
                    [P, 2, Tp], bf, tag="qkt", name=f"qkt_{u}"
                )
            qt = qkt_tiles[u]
            m = u if half == 0 else M2C // 2 + u
            msl = slice(m * P, (m + 1) * P)
            tsl = slice(tj * TCH, (tj + 1) * TCH)
            ps = psmm.tile([P, TCH], f32, tag="mm")
            for ct in range(CT):
                nc.tensor.matmul(
                    ps[:], lhsT=wqk[:, ct, msl], rhs=xT[:, ct, tsl],
                    start=(ct == 0), stop=(ct == CT - 1),
                )
            nc.vector.tensor_scalar_add(qt[:, half, tsl], ps[:], bqk[:, m:m + 1])

        def _emit_qk_unit(u):
            for half in (0, 1):
                for tj in range(TJ):
                    _emit_qk_chain(u, half, tj)

        def _emit_v_chunk(ti, vj):
            tsl = slice(ti * P, (ti + 1) * P)
            vsl = slice(vj * QC, (vj + 1) * QC)
            ps = psmm.tile([P, QC], f32, tag="mm")
            for ct in range(CT):
                nc.tensor.matmul(
                    ps[:], lhsT=xT[:, ct, tsl], rhs=wv[:, ct, vsl],
                    start=(ct == 0),
                    stop=(ct == CT - 1 and not with_bias_v),
                )
            if with_bias_v:
                nc.tensor.matmul(
                    ps[:], lhsT=ones[0:1, 0:P], rhs=bv[0:1, vsl],
                    start=False, stop=True,
                )
            hpc = QC // HDp  # heads per chunk
            nc.vector.tensor_copy(
                out=Vp[:, ti, vj * hpc:(vj + 1) * hpc, HDp:VW],
                in_=ps[:].rearrange("p (h d) -> p h d", d=HDp),
            )

        # ---- attention pipeline pieces ----
        # Head pairs: head 2u on partitions 0:64, head 2u+1 on 64:128 of
        # qkT chunk u (q) / M2C//2+u (k). The A/B matmuls use disjoint PE
        # row groups (tile_position auto-derived from base_partition), so
        # they run concurrently in the array.
        E_tiles = {}

        def _ecols(qj):
            nk = DIAG * (qj + 1)
            return nk * QC

        def _emit_S(it):
            u, qj = divmod(it, NQ)
            nk = DIAG * (qj + 1)
            q0 = qj * QC
            pool = epool0 if qj == 0 else epool1
            E_A = pool.tile([P, _ecols(qj)], bf, tag=f"E{qj}")
            E_B = pool.tile([P, _ecols(qj)], bf, tag=f"E{qj}")
            E_tiles[it] = (E_A, E_B)
            qt = qkt_tiles[u]
            qk_parts = (
                (qt[0:HDp, 0, :], qt[0:HDp, 1, :]),
                (qt[HDp:P, 0, :], qt[HDp:P, 1, :]),
            )
            for g in range(nk // 2):
                offg = max(0, P * (2 * g - DIAG * qj))
                ps_h = [
                    psatt.tile([P, 2, QC], f32, tag="satt",
                               name=f"ps_s_{u}_{qj}_{g}_{hh}")
                    for hh in range(2)
                ]
                for r2 in range(2):
                    ki = 2 * g + r2
                    ksl = slice(ki * P, (ki + 1) * P)
                    for half, (qT, kT) in enumerate(qk_parts):
                        nc.tensor.matmul(
                            ps_h[half][:, r2, offg:],
                            lhsT=kT[:, ksl],
                            rhs=qT[:, q0 + offg:q0 + QC],
                            start=True, stop=True,
                        )
                for half, E in enumerate((E_A, E_B)):
                    eap = E[:, 2 * g * QC: (2 * g + 2) * QC].rearrange(
                        "p (a q) -> p a q", a=2
                    )[:, :, offg:]
                    nc.scalar.activation(eap, ps_h[half][:, :, offg:], AF.Exp)
            # causal mask: strided-AP multiplies over the DIAG diagonal
            # 128-col blocks (stride QC+P), on DVE (no cross-engine waits
            # beyond exp, so the DVE FIFO never blocks on GpSimd).
            # Split 3+1 so the rearrange slice stays inside the E tile.
            dbase = DIAG * qj * QC
            for E in (E_A, E_B):
                dap = E[:, dbase: dbase + (DIAG - 1) * (QC + P)].rearrange(
                    "p (r j) -> p r j", r=DIAG - 1
                )[:, :, 0:P]
                nc.vector.tensor_mul(out=dap, in0=dap, in1=masks[:, 0:DIAG - 1, :])
                last = dbase + (DIAG - 1) * (QC + P)
                nc.vector.tensor_mul(
                    out=E[:, last: last + P], in0=E[:, last: last + P],
                    in1=masks[:, DIAG - 1, :],
                )

        tails = {}

        def _emit_tail_av(it):
            u, qj = divmod(it, NQ)
            nk = DIAG * (qj + 1)
            E_A, E_B = E_tiles.pop(it)
            # O'[d,q] at partitions 64:128 + sums row at partition 0, via the
            # [ones|pad|v] columns of Vp; A and B into the two bank-slots of
            # one PSUM tile
            pav = psatt.tile([P, 2, QC], f32, tag="satt",
                             name=f"pav_{u}_{qj}")
            for g, E, h in ((0, E_A, 2 * u), (1, E_B, 2 * u + 1)):
                for ki in range(nk):
                    off = max(0, P * (ki - DIAG * qj))
                    nc.tensor.matmul(
                        pav[:, g, off:],
                        lhsT=Vp[:, ki, h, :],
                        rhs=E[:, ki * QC + off: (ki + 1) * QC],
                        start=(ki == 0), stop=(ki == nk - 1),
                    )
            rrow = rpool2.tile([1, 2, QC], f32, tag="rrow")
            rb = rpool2.tile([HDp, 2, QC], f32, tag="rb")
            nc.vector.reciprocal_approx_fast(rrow[:], pav[0:1, :, :])
            # GpSimd runs ONLY partition_broadcast, so its ucode library
            # never swaps (each swap costs ~6us of hidden latency)
            nc.gpsimd.partition_broadcast(rb[:], rrow[:])
            tails[it] = (pav, rb)

        def _emit_tail_mul(it):
            # normalize multiplies, deferred one iteration so this DVE work
            # never waits on the GpSimd broadcast at the FIFO head
            u, qj = divmod(it, NQ)
            pav, rb = tails.pop(it)
            qsl = slice(qj * QC, qj * QC + QC)
            nc.vector.tensor_mul(
                out=YT[0:HDp, u, qsl], in0=pav[HDp:P, 0, :], in1=rb[:, 0, :],
            )
            nc.vector.tensor_mul(
                out=YT[HDp:P, u, qsl], in0=pav[HDp:P, 1, :], in1=rb[:, 1, :],
            )

        def _emit_proj_chunk(ti, zj):
            tsl = slice(ti * P, (ti + 1) * P)
            zsl = slice(zj * QC, (zj + 1) * QC)
            ps = psmm.tile([P, QC], f32, tag="mm")
            for ct in range(CT):
                nc.tensor.matmul(
                    ps[:], lhsT=YT[:, ct, tsl], rhs=wp[:, ct, zsl],
                    start=(ct == 0),
                    stop=(ct == CT - 1 and not with_bias_p),
                )
            if with_bias_p:
                nc.tensor.matmul(
                    ps[:], lhsT=ones[0:1, 0:P], rhs=bp[0:1, zsl],
                    start=False, stop=True,
                )
            zt = zpool.tile([P, QC], f32, tag="zt")
            nc.vector.tensor_copy(out=zt[:], in_=ps[:])
            nc.sync.dma_start(out_d[tsl, zsl], zt[:])

        def _emit_body():
            qkt_tiles.clear()
            # prologue: qkT for units 0-1, S(0), V interleaved with S(1)
            _emit_qk_unit(0)
            _emit_qk_unit(1)
            _emit_S(0)
            for ti in range(TT // 2):
                for vj in range(VJ):
                    _emit_v_chunk(ti, vj)
            _emit_S(1)
            for ti in range(TT // 2, TT):
                for vj in range(VJ):
                    _emit_v_chunk(ti, vj)
            _emit_tail_av(0)
            # steady state: S one iteration ahead of the AV tail, normalize
            # multiplies one further behind, two qkT chains per iteration as
            # PE filler. Unit u's chains finish emitting before S(2u).
            fill = []
            for u in range(2, NU):
                for half in (0, 1):
                    for tj in range(TJ):
                        fill.append((u, half, tj))
            nfill = len(fill)
            fi = 0
            _emit_tail_mul(0)
            for it in range(2, NIT):
                _emit_S(it)
                _emit_tail_av(it - 1)
                _emit_tail_mul(it - 1)
                for _ in range(min(2, nfill - fi)):
                    _emit_qk_chain(*fill[fi])
                    fi += 1
            while fi < nfill:
                _emit_qk_chain(*fill[fi])
                fi += 1
            _emit_tail_av(NIT - 1)
            _emit_tail_mul(NIT - 1)
            for ti in range(TT):
                for zj in range(VJ):
                    _emit_proj_chunk(ti, zj)

        if reps == 1:
            _emit_body()
        else:
            hint = (
                mybir.EngineType.PE,
                mybir.EngineType.DVE,
                mybir.EngineType.Activation,
            )
            with tc.For_i(0, reps, 1, hint_engines=hint):
                _emit_body()

    nc.finalize()
    return nc


def _prep_shared(w_attn, b_attn, w_proj, b_proj):
    """Host-side layout marshalling of the replicated weights (bf16 cast,
    per-head q/k/v column gather, exact 1/8 q pre-scale)."""
    wr = np.asarray(w_attn, np.float32).reshape(C, H, 3, HD)
    w_q = (wr[:, :, 0, :] * np.float32(0.125)).reshape(C, C)
    w_k = wr[:, :, 1, :].reshape(C, C)
    w_qk = np.ascontiguousarray(
        np.concatenate([w_q, w_k], axis=1)
    ).astype(BF16)
    w_v = np.ascontiguousarray(wr[:, :, 2, :].reshape(C, C)).astype(BF16)

    br = np.asarray(b_attn, np.float32).reshape(H, 3, HD)
    # per-partition column layout for the qkT copyback bias: [M2C, 128] f32
    b_qk = np.ascontiguousarray(
        np.concatenate(
            [(br[:, 0, :] * np.float32(0.125)).reshape(C), br[:, 1, :].reshape(C)]
        ).reshape(2 * C // 128, 128)
    )
    b_v = np.ascontiguousarray(br[:, 2, :].reshape(1, C)).astype(BF16)

    wp = np.ascontiguousarray(np.asarray(w_proj, np.float32)).astype(BF16)
    bp = np.ascontiguousarray(np.asarray(b_proj, np.float32).reshape(1, C)).astype(BF16)

    # lower-tri 128x128 (k <= q within a diagonal block), replicated DIAG
    # times to match the strided diagonal-block AP
    DIAGv = min(512, T) // 128
    k_idx = np.arange(128)[:, None]
    q_idx = np.arange(128)[None, :]
    tri = (k_idx <= q_idx)
    masks = np.ascontiguousarray(
        np.broadcast_to(tri[:, None, :], (128, DIAGv, 128))
    ).astype(BF16)
    return w_qk, w_v, wp, b_qk, b_v, bp, masks


class _Runner:
    """Cached jit(shard_map) executor for a prebuilt Bass module across
    N cores — same lowering as bass2jax.run_bass_via_pjrt, but reusable
    across calls so warm executions can be timed."""

    def __init__(self, nc, n_cores):
        import jax
        import numpy as _np
        from jax.sharding import Mesh, PartitionSpec
        try:
            from jax.experimental.shard_map import shard_map
        except ImportError:
            from jax.shard_map import shard_map
        from concourse import bass2jax, mybir

        bass2jax.install_neuronx_cc_hook()
        assert not nc.dbg_callbacks
        self.dbg_name = nc.dbg_addr.name if nc.dbg_addr is not None else None
        partition_name = (
            nc.partition_id_tensor.name if nc.partition_id_tensor else None
        )

        in_names, out_names, out_avals = [], [], []
        for alloc in nc.m.functions[0].allocations:
            if not isinstance(alloc, mybir.MemoryLocationSet):
                continue
            name = alloc.memorylocations[0].name
            if alloc.kind == "ExternalInput":
                if name != partition_name:
                    in_names.append(name)
            elif alloc.kind == "ExternalOutput":
                out_names.append(name)
                out_avals.append(
                    jax.core.ShapedArray(
                        tuple(alloc.tensor_shape), mybir.dt.np(alloc.dtype)
                    )
                )
        self.n_params = len(in_names)
        self.in_names = list(in_names)
        self.out_names = out_names
        self.out_avals = out_avals
        self.n_cores = n_cores
        all_names = in_names + out_names
        if partition_name is not None:
            all_names = all_names + [partition_name]

        def _body(*args):
            operands = list(args)
            if partition_name is not None:
                operands.append(bass2jax.partition_id_tensor())
            outs = bass2jax._bass_exec_p.bind(
                *operands,
                out_avals=tuple(out_avals),
                in_names=tuple(all_names),
                out_names=tuple(out_names),
                lowering_input_output_aliases=(),
                sim_require_finite=True,
                sim_require_nnan=True,
                nc=nc,
            )
            return tuple(outs)

        devices = jax.devices()[:n_cores]
        mesh = Mesh(_np.asarray(devices), ("core",))
        n_outs = len(out_names)
        # No donation: the kernel writes every element of every output, so
        # the zero "output seed" operands can live on device and be reused
        # across timed calls.
        self.jitted = jax.jit(
            shard_map(
                _body,
                mesh=mesh,
                in_specs=(PartitionSpec("core"),) * (self.n_params + n_outs),
                out_specs=(PartitionSpec("core"),) * n_outs,
                check_rep=False,
            ),
            keep_unused=True,
        )
        from jax.sharding import NamedSharding

        self.sharding = NamedSharding(mesh, PartitionSpec("core"))
        self.dev_zeros = [
            jax.device_put(
                _np.zeros((n_cores * a.shape[0], *a.shape[1:]), a.dtype),
                self.sharding,
            )
            for a in out_avals
        ]

    def prep_args(self, in_maps):
        import jax
        import numpy as _np

        if self.dbg_name is not None:
            # 8-byte PA as uint32[1,2]; zero -> debugger store+halt skipped
            dbg = _np.zeros((1, 2), _np.uint32)
            in_maps = [{**m, self.dbg_name: dbg} for m in in_maps]
        return [
            jax.device_put(
                _np.concatenate(
                    [_np.asarray(m[name]) for m in in_maps], axis=0
                ),
                self.sharding,
            )
            for name in self.in_names
        ]

    def run(self, concat_in):
        import jax

        out = self.jitted(*concat_in, *self.dev_zeros)
        return jax.block_until_ready(out)

    def results(self, out_arrs):
        import numpy as _np

        return [
            {
                name: _np.asarray(out_arrs[i]).reshape(
                    self.n_cores, *self.out_avals[i].shape
                )[c]
                for i, name in enumerate(self.out_names)
            }
            for c in range(self.n_cores)
        ]


_RUNNER_CACHE = {}


def _get_runner(reps=1, with_bias_v=True, with_bias_p=True):
    key = (T, C, H, HD, reps, with_bias_v, with_bias_p)
    if key not in _RUNNER_CACHE:
        if key not in _NC_CACHE:
            _NC_CACHE[key] = _build_nc(
                T, C, H, HD, reps=reps,
                with_bias_v=with_bias_v, with_bias_p=with_bias_p,
            )
        _RUNNER_CACHE[key] = _Runner(_NC_CACHE[key], N_CORES)
    return _RUNNER_CACHE[key]


def _make_in_maps(x, w_attn, b_attn, w_proj, b_proj):
    w_qk, w_v, wp, b_qk, b_v, bp, masks = _prep_shared(
        w_attn, b_attn, w_proj, b_proj
    )
    x = np.asarray(x, np.float32)
    in_maps = []
    for i in range(N_CORES):
        xT = np.ascontiguousarray(x[i].T).astype(BF16)
        in_maps.append({
            "xT": xT, "w_qk": w_qk, "w_v": w_v, "w_proj": wp,
            "b_qk": b_qk, "b_v": b_v, "b_proj": bp, "masks": masks,
        })
    return in_maps


def _bias_flags(b_attn, b_proj):
    bv = np.asarray(b_attn, np.float32).reshape(H, 3, HD)[:, 2, :]
    return bool(bv.any()), bool(np.asarray(b_proj, np.float32).any())


def kernel(x, w_attn, b_attn, w_proj, b_proj):
    wbv, wbp = _bias_flags(b_attn, b_proj)
    runner = _get_runner(with_bias_v=wbv, with_bias_p=wbp)
    concat_in = runner.prep_args(
        _make_in_maps(x, w_attn, b_attn, w_proj, b_proj)
    )
    res = runner.results(runner.run(concat_in))
    return np.stack([res[i]["out"] for i in range(N_CORES)]).astype(np.float32)


def measure(x, w_attn, b_attn, w_proj, b_proj, iters=5, reps=1):
    """Warm wall-clock times (s) of the sharded on-device execution.

    reps > 1 uses a kernel variant whose compute body runs `reps` times
    per dispatch (device-side loop), so per-iteration HW time can be
    resolved despite the ~90 ms axon round-trip overhead."""
    import time

    wbv, wbp = _bias_flags(b_attn, b_proj)
    runner = _get_runner(reps=reps, with_bias_v=wbv, with_bias_p=wbp)
    concat_in = runner.prep_args(
        _make_in_maps(x, w_attn, b_attn, w_proj, b_proj)
    )
    runner.run(concat_in)  # warm-up / compile
    times = []
    for _ in range(iters):
        t0 = time.perf_counter()
        runner.run(concat_in)
        times.append(time.perf_counter() - t0)
    return times


# revision 54
# speedup vs baseline: 1.0103x; 1.0103x over previous
"""Causal self-attention on 8 TRN2 NeuronCores.

Sharding: pure data-parallel on batch (B=8 -> one batch element per core,
no collectives). Each core computes its full [T, C] output slice.

Per-core dataflow (all matmuls bf16 with fp32 PSUM accumulation):
  xT [C,T] (host-pretransposed) --+--> qkT = (w_qk^T @ xT) + b_qk  [2C, T]
                                  +--> V   = (xT^T @ w_v) + b_v    [T, C] (padded
                                       with a ones column per head: [.., 65])
  per head-pair u, per q-chunk (512):
    S^T[k,q] = k_h . q_h           (lhsT = kT_h slice, rhs = qT_h slice; the
                                    A/B head halves run concurrently in
                                    disjoint PE row groups)
    E = exp(S^T)                   (ACT, PSUM->SBUF, bf16 out; q pre-scaled
                                    by 1/8 on host so no separate scale op)
    E *= causal mask               (single strided-AP multiply covering only
                                    the 128-col diagonal blocks, on GpSimd)
    O'[d,q], s[q] = [v_h | 1]^T @ E  (augmented-ones row gives softmax sums;
                                      A and B go to slots of one PSUM tile)
    r = 1/s (DVE reciprocal_approx_fast, f32)
    R = partition_broadcast(r)     (GpSimd, SBUF->SBUF)
    Y^T block = O' * R             (DVE)
  Z = (Y^T)^T @ w_proj + b_proj    -> DMA PSUM -> DRAM out

Scheduling: the qkT GEMM is split per head-pair unit and its chains are
interleaved into the attention pipeline as PE filler, so the tensor engine
streams continuously (keeps the HAM clock-gate at 8/8) while exp/recip/
normalize latencies hide underneath. Input DMAs are split so the first
matmul can start ~2us in. Biases enter as K=1 rank-1 accumulate matmuls.
"""

import os
import sys
from contextlib import ExitStack

import numpy as np

try:
    import ml_dtypes
except ImportError:  # pragma: no cover
    sys.path.insert(0, "/opt/trn_rl_repo")
    import ml_dtypes

BF16 = ml_dtypes.bfloat16

B, T, C = 8, 1024, 1024
H, HD = 16, 64
N_CORES = 8

# Toggled by test harness to capture a hardware profile.
TRACE = False
LAST_EXEC_NS = None
LAST_RESULTS = None

_NC_CACHE = {}


def _build_nc(Tp, Cp, Hp, HDp, reps=1, with_bias_v=True, with_bias_p=True):
    import concourse.bass as bass
    import concourse.tile as tile
    from concourse import bacc, mybir

    bf = mybir.dt.bfloat16
    f32 = mybir.dt.float32
    AF = mybir.ActivationFunctionType

    P = 128
    CT = Cp // P            # c-tiles (contraction tiles)
    TT = Tp // P            # t-tiles
    QC = min(512, Tp)       # q-chunk width (free dim per matmul)
    NQ = Tp // QC           # q-chunks
    TCH = min(512, Tp)      # t-chunk width for qkT rhs
    TJ = Tp // TCH
    DIAG = QC // P          # diagonal k-tiles per q-chunk
    M2C = 2 * Cp // P       # qk m-chunks
    VJ = Cp // QC           # v/proj column chunks
    NU = Hp // 2            # head-pair units
    NIT = NU * NQ           # attention iterations (unit, q-chunk)

    nc = bacc.Bacc("TRN2", target_bir_lowering=False, debug=False)

    xT_d = nc.declare_dram_parameter("xT", [Cp, Tp], bf, isOutput=False)
    wqk_d = nc.declare_dram_parameter("w_qk", [Cp, 2 * Cp], bf, isOutput=False)
    wv_d = nc.declare_dram_parameter("w_v", [Cp, Cp], bf, isOutput=False)
    wp_d = nc.declare_dram_parameter("w_proj", [Cp, Cp], bf, isOutput=False)
    bqk_d = nc.declare_dram_parameter("b_qk", [M2C, P], f32, isOutput=False)
    bv_d = nc.declare_dram_parameter("b_v", [1, Cp], bf, isOutput=False)
    bp_d = nc.declare_dram_parameter("b_proj", [1, Cp], bf, isOutput=False)
    # lower-tri [k<=q] 128x128 block, replicated DIAG times along the middle
    # axis to match the strided diagonal-block AP of the E tiles
    mask_d = nc.declare_dram_parameter("masks", [P, DIAG, P], bf, isOutput=False)
    out_d = nc.declare_dram_parameter("out", [Tp, Cp], f32, isOutput=True)

    with tile.TileContext(nc) as tc, ExitStack() as ctx:
        consts = ctx.enter_context(tc.tile_pool(name="consts", bufs=1))
        # E tiles are flat [P, cols]; qj=0 needs 2560 cols, qj=1 needs 4608
        epool0 = ctx.enter_context(tc.tile_pool(name="epool0", bufs=2))
        epool1 = ctx.enter_context(tc.tile_pool(name="epool1", bufs=2))
        rpool2 = ctx.enter_context(tc.tile_pool(name="rpool2", bufs=2))
        zpool = ctx.enter_context(tc.tile_pool(name="zpool", bufs=2))
        # qkT head-pair tiles rotate through 4 slots (live qk-emit -> S use)
        qkpool = ctx.enter_context(tc.tile_pool(name="qkpool", bufs=4))
        # S-pair tiles and the combined AV accumulator share one 3-slot pool
        # (each [P, 2, QC] f32 = 2 banks -> 6 banks) + 2 banks for GEMM chains
        psatt = ctx.enter_context(tc.tile_pool(name="psatt", bufs=3, space="PSUM"))
        psmm = ctx.enter_context(tc.tile_pool(name="psmm", bufs=2, space="PSUM"))

        # ---- persistent SBUF buffers ----
        xT = consts.tile([P, CT, Tp], bf)
        wqk = consts.tile([P, CT, 2 * Cp], bf)
        wv = consts.tile([P, CT, Cp], bf)
        wp = consts.tile([P, CT, Cp], bf)
        # per head: [ones | 63 pad | v0..v63] (128 cols) -> softmax-sum row
        # lands at PSUM partition 0 (where the custom-DVE reciprocal reads)
        # and O' at partitions 64:128 (legal 64-partition base)
        VW = 2 * HDp
        Vp = consts.tile([P, TT, Hp, VW], bf)
        YT = consts.tile([P, CT, Tp], bf)
        bqk = consts.tile([P, M2C], f32)
        bv = bp = None
        if with_bias_v:
            bv = consts.tile([1, Cp], bf)
        if with_bias_p:
            bp = consts.tile([1, Cp], bf)
        ones = None
        if with_bias_v or with_bias_p:
            ones = consts.tile([1, max(QC, P)], bf)
        masks = consts.tile([P, DIAG, P], bf)

        # ---- input DMAs, priority-ordered and split for early PE start ----
        # wqk column pairs: unit u needs cols [128u,128u+128) and Cp+[...]
        def _dma_wqk(lo_u, hi_u):
            w = (hi_u - lo_u) * P
            for base in (0, Cp):
                nc.sync.dma_start(
                    wqk[:, :, base + lo_u * P: base + lo_u * P + w],
                    wqk_d[:, base + lo_u * P: base + lo_u * P + w].rearrange(
                        "(ct p) n -> p ct n", p=P
                    ),
                )

        # Each dma_start costs ~1.1us of serial issue on the sync engine, so
        # the first qk chain's dependencies (bqk, wqk unit 0, xT) are issued
        # first and with minimal instruction count (xT in 2-ct chunks).
        nc.sync.dma_start(bqk[:], bqk_d.rearrange("m p -> p m"))
        _dma_wqk(0, 1)  # unit 0 only
        for ct in range(0, CT, 2):
            nc.sync.dma_start(
                xT[:, ct:ct + 2, :],
                xT_d[ct * P: (ct + 2) * P, :].rearrange(
                    "(a p) t -> p a t", p=P
                ),
            )
        _dma_wqk(1, 2)
        nc.sync.dma_start(wv[:], wv_d.rearrange("(ct p) n -> p ct n", p=P))
        nc.sync.dma_start(masks[:], mask_d[:])
        _dma_wqk(2, NU)  # remaining units
        nc.sync.dma_start(wp[:], wp_d.rearrange("(ct p) n -> p ct n", p=P))
        if with_bias_v:
            nc.sync.dma_start(bv[:], bv_d[:])
        if with_bias_p:
            nc.sync.dma_start(bp[:], bp_d[:])
        if ones is not None:
            nc.gpsimd.memset(ones[:], 1.0)
        nc.gpsimd.memset(Vp[:], 1.0)  # ones column survives; rest overwritten

        qkt_tiles = {}

        def _emit_qk_chain(u, half, tj):
            # one qkT chain: [P, TCH] psum accumulated over CT, then bias+copy
            # into slot `half` (0=q, 1=k) of unit u's rotating qkT tile
            if u not in qkt_tiles:
                qkt_tiles[u] = qkpool.tile(
                    [P, 2, Tp], bf, tag="qkt", name=f"qkt_{u}"
                )
            qt = qkt_tiles[u]
            m = u if half == 0 else M2C // 2 + u
            msl = slice(m * P, (m + 1) * P)
            tsl = slice(tj * TCH, (tj + 1) * TCH)
            ps = psmm.tile([P, TCH], f32, tag="mm")
            for ct in range(CT):
                nc.tensor.matmul(
                    ps[:], lhsT=wqk[:, ct, msl], rhs=xT[:, ct, tsl],
                    start=(ct == 0), stop=(ct == CT - 1),
                )
            nc.vector.tensor_scalar_add(qt[:, half, tsl], ps[:], bqk[:, m:m + 1])

        def _emit_qk_unit(u):
            for half in (0, 1):
                for tj in range(TJ):
                    _emit_qk_chain(u, half, tj)

        def _emit_v_chunk(ti, vj):
            tsl = slice(ti * P, (ti + 1) * P)
            vsl = slice(vj * QC, (vj + 1) * QC)
            ps = psmm.tile([P, QC], f32, tag="mm")
            for ct in range(CT):
                nc.tensor.matmul(
                    ps[:], lhsT=xT[:, ct, tsl], rhs=wv[:, ct, vsl],
                    start=(ct == 0),
                    stop=(ct == CT - 1 and not with_bias_v),
                )
            if with_bias_v:
                nc.tensor.matmul(
                    ps[:], lhsT=ones[0:1, 0:P], rhs=bv[0:1, vsl],
                    start=False, stop=True,
                )
            hpc = QC // HDp  # heads per chunk
            nc.vector.tensor_copy(
                out=Vp[:, ti, vj * hpc:(vj + 1) * hpc, HDp:VW],
                in_=ps[:].rearrange("p (h d) -> p h d", d=HDp),
            )

        # ---- attention pipeline pieces ----
        # Head pairs: head 2u on partitions 0:64, head 2u+1 on 64:128 of
        # qkT chunk u (q) / M2C//2+u (k). The A/B matmuls use disjoint PE
        # row groups (tile_position auto-derived from base_partition), so
        # they run concurrently in the array.
        E_tiles = {}

        def _ecols(qj):
            nk = DIAG * (qj + 1)
            return nk * QC

        def _emit_S(it):
            u, qj = divmod(it, NQ)
            nk = DIAG * (qj + 1)
            q0 = qj * QC
            pool = epool0 if qj == 0 else epool1
            E_A = pool.tile([P, _ecols(qj)], bf, tag=f"E{qj}")
            E_B = pool.tile([P, _ecols(qj)], bf, tag=f"E{qj}")
            E_tiles[it] = (E_A, E_B)
            qt = qkt_tiles[u]
            qk_parts = (
                (qt[0:HDp, 0, :], qt[0:HDp, 1, :]),
                (qt[HDp:P, 0, :], qt[HDp:P, 1, :]),
            )
            for g in range(nk // 2):
                offg = max(0, P * (2 * g - DIAG * qj))
                ps_h = [
                    psatt.tile([P, 2, QC], f32, tag="satt",
                               name=f"ps_s_{u}_{qj}_{g}_{hh}")
                    for hh in range(2)
                ]
                for r2 in range(2):
                    ki = 2 * g + r2
                    ksl = slice(ki * P, (ki + 1) * P)
                    for half, (qT, kT) in enumerate(qk_parts):
                        nc.tensor.matmul(
                            ps_h[half][:, r2, offg:],
                            lhsT=kT[:, ksl],
                            rhs=qT[:, q0 + offg:q0 + QC],
                            start=True, stop=True,
                        )
                for half, E in enumerate((E_A, E_B)):
                    eap = E[:, 2 * g * QC: (2 * g + 2) * QC].rearrange(
                        "p (a q) -> p a q", a=2
                    )[:, :, offg:]
                    nc.scalar.activation(eap, ps_h[half][:, :, offg:], AF.Exp)
            # causal mask: strided-AP multiplies over the DIAG diagonal
            # 128-col blocks (stride QC+P), on DVE (no cross-engine waits
            # beyond exp, so the DVE FIFO never blocks on GpSimd).
            # Split 3+1 so the rearrange slice stays inside the E tile.
            dbase = DIAG * qj * QC
            for E in (E_A, E_B):
                dap = E[:, dbase: dbase + (DIAG - 1) * (QC + P)].rearrange(
                    "p (r j) -> p r j", r=DIAG - 1
                )[:, :, 0:P]
                nc.vector.tensor_mul(out=dap, in0=dap, in1=masks[:, 0:DIAG - 1, :])
                last = dbase + (DIAG - 1) * (QC + P)
                nc.vector.tensor_mul(
                    out=E[:, last: last + P], in0=E[:, last: last + P],
                    in1=masks[:, DIAG - 1, :],
                )

        tails = {}

        def _emit_tail_av(it):
            u, qj = divmod(it, NQ)
            nk = DIAG * (qj + 1)
            E_A, E_B = E_tiles.pop(it)
            # O'[d,q] at partitions 64:128 + sums row at partition 0, via the
            # [ones|pad|v] columns of Vp; A and B into the two bank-slots of
            # one PSUM tile
            pav = psatt.tile([P, 2, QC], f32, tag="satt",
                             name=f"pav_{u}_{qj}")
            for g, E, h in ((0, E_A, 2 * u), (1, E_B, 2 * u + 1)):
                for ki in range(nk):
                    off = max(0, P * (ki - DIAG * qj))
                    nc.tensor.matmul(
                        pav[:, g, off:],
                        lhsT=Vp[:, ki, h, :],
                        rhs=E[:, ki * QC + off: (ki + 1) * QC],
                        start=(ki == 0), stop=(ki == nk - 1),
                    )
            rrow = rpool2.tile([1, 2, QC], f32, tag="rrow")
            rb = rpool2.tile([HDp, 2, QC], f32, tag="rb")
            nc.vector.reciprocal_approx_fast(rrow[:], pav[0:1, :, :])
            # GpSimd runs ONLY partition_broadcast, so its ucode library
            # never swaps (each swap costs ~6us of hidden latency)
            nc.gpsimd.partition_broadcast(rb[:], rrow[:])
            tails[it] = (pav, rb)

        def _emit_tail_mul(it):
            # normalize multiplies, deferred one iteration so this DVE work
            # never waits on the GpSimd broadcast at the FIFO head
            u, qj = divmod(it, NQ)
            pav, rb = tails.pop(it)
            qsl = slice(qj * QC, qj * QC + QC)
            nc.vector.tensor_mul(
                out=YT[0:HDp, u, qsl], in0=pav[HDp:P, 0, :], in1=rb[:, 0, :],
            )
            nc.vector.tensor_mul(
                out=YT[HDp:P, u, qsl], in0=pav[HDp:P, 1, :], in1=rb[:, 1, :],
            )

        def _emit_proj_chunk(ti, zj):
            tsl = slice(ti * P, (ti + 1) * P)
            zsl = slice(zj * QC, (zj + 1) * QC)
            ps = psmm.tile([P, QC], f32, tag="mm")
            for ct in range(CT):
                nc.tensor.matmul(
                    ps[:], lhsT=YT[:, ct, tsl], rhs=wp[:, ct, zsl],
                    start=(ct == 0),
                    stop=(ct == CT - 1 and not with_bias_p),
                )
            if with_bias_p:
                nc.tensor.matmul(
                    ps[:], lhsT=ones[0:1, 0:P], rhs=bp[0:1, zsl],
                    start=False, stop=True,
                )
            zt = zpool.tile([P, QC], f32, tag="zt")
            nc.vector.tensor_copy(out=zt[:], in_=ps[:])
            nc.sync.dma_start(out_d[tsl, zsl], zt[:])

        def _emit_body():
            qkt_tiles.clear()
            # prologue: qkT for units 0-1, S(0), V interleaved with S(1)
            _emit_qk_unit(0)
            _emit_qk_unit(1)
            _emit_S(0)
            for ti in range(TT // 2):
                for vj in range(VJ):
                    _emit_v_chunk(ti, vj)
            _emit_S(1)
            for ti in range(TT // 2, TT):
                for vj in range(VJ):
                    _emit_v_chunk(ti, vj)
            _emit_tail_av(0)
            # steady state: S one iteration ahead of the AV tail, normalize
            # multiplies one further behind, two qkT chains per iteration as
            # PE filler. Unit u's chains finish emitting before S(2u).
            fill = []
            for u in range(2, NU):
                for half in (0, 1):
                    for tj in range(TJ):
                        fill.append((u, half, tj))
            nfill = len(fill)
            fi = 0
            _emit_tail_mul(0)
            for it in range(2, NIT):
                _emit_S(it)
                _emit_tail_av(it - 1)
                _emit_tail_mul(it - 1)
                for _ in range(min(2, nfill - fi)):
                    _emit_qk_chain(*fill[fi])
                    fi += 1
            while fi < nfill:
                _emit_qk_chain(*fill[fi])
                fi += 1
            _emit_tail_av(NIT - 1)
            _emit_tail_mul(NIT - 1)
            for ti in range(TT):
                for zj in range(VJ):
                    _emit_proj_chunk(ti, zj)

        if reps == 1:
            _emit_body()
        else:
            hint = (
                mybir.EngineType.PE,
                mybir.EngineType.DVE,
                mybir.EngineType.Activation,
            )
            with tc.For_i(0, reps, 1, hint_engines=hint):
                _emit_body()

    nc.finalize()
    return nc


def _prep_shared(w_attn, b_attn, w_proj, b_proj):
    """Host-side layout marshalling of the replicated weights (bf16 cast,
    per-head q/k/v column gather, exact 1/8 q pre-scale)."""
    wr = np.asarray(w_attn, np.float32).reshape(C, H, 3, HD)
    w_q = (wr[:, :, 0, :] * np.float32(0.125)).reshape(C, C)
    w_k = wr[:, :, 1, :].reshape(C, C)
    w_qk = np.ascontiguousarray(
        np.concatenate([w_q, w_k], axis=1)
    ).astype(BF16)
    w_v = np.ascontiguousarray(wr[:, :, 2, :].reshape(C, C)).astype(BF16)

    br = np.asarray(b_attn, np.float32).reshape(H, 3, HD)
    # per-partition column layout for the qkT copyback bias: [M2C, 128] f32
    b_qk = np.ascontiguousarray(
        np.concatenate(
            [(br[:, 0, :] * np.float32(0.125)).reshape(C), br[:, 1, :].reshape(C)]
        ).reshape(2 * C // 128, 128)
    )
    b_v = np.ascontiguousarray(br[:, 2, :].reshape(1, C)).astype(BF16)

    wp = np.ascontiguousarray(np.asarray(w_proj, np.float32)).astype(BF16)
    bp = np.ascontiguousarray(np.asarray(b_proj, np.float32).reshape(1, C)).astype(BF16)

    # lower-tri 128x128 (k <= q within a diagonal block), replicated DIAG
    # times to match the strided diagonal-block AP
    DIAGv = min(512, T) // 128
    k_idx = np.arange(128)[:, None]
    q_idx = np.arange(128)[None, :]
    tri = (k_idx <= q_idx)
    masks = np.ascontiguousarray(
        np.broadcast_to(tri[:, None, :], (128, DIAGv, 128))
    ).astype(BF16)
    return w_qk, w_v, wp, b_qk, b_v, bp, masks


class _Runner:
    """Cached jit(shard_map) executor for a prebuilt Bass module across
    N cores — same lowering as bass2jax.run_bass_via_pjrt, but reusable
    across calls so warm executions can be timed."""

    def __init__(self, nc, n_cores):
        import jax
        import numpy as _np
        from jax.sharding import Mesh, PartitionSpec
        try:
            from jax.experimental.shard_map import shard_map
        except ImportError:
            from jax.shard_map import shard_map
        from concourse import bass2jax, mybir

        bass2jax.install_neuronx_cc_hook()
        assert not nc.dbg_callbacks
        self.dbg_name = nc.dbg_addr.name if nc.dbg_addr is not None else None
        partition_name = (
            nc.partition_id_tensor.name if nc.partition_id_tensor else None
        )

        in_names, out_names, out_avals = [], [], []
        for alloc in nc.m.functions[0].allocations:
            if not isinstance(alloc, mybir.MemoryLocationSet):
                continue
            name = alloc.memorylocations[0].name
            if alloc.kind == "ExternalInput":
                if name != partition_name:
                    in_names.append(name)
            elif alloc.kind == "ExternalOutput":
                out_names.append(name)
                out_avals.append(
                    jax.core.ShapedArray(
                        tuple(alloc.tensor_shape), mybir.dt.np(alloc.dtype)
                    )
                )
        self.n_params = len(in_names)
        self.in_names = list(in_names)
        self.out_names = out_names
        self.out_avals = out_avals
        self.n_cores = n_cores
        all_names = in_names + out_names
        if partition_name is not None:
            all_names = all_names + [partition_name]

        def _body(*args):
            operands = list(args)
            if partition_name is not None:
                operands.append(bass2jax.partition_id_tensor())
            outs = bass2jax._bass_exec_p.bind(
                *operands,
                out_avals=tuple(out_avals),
                in_names=tuple(all_names),
                out_names=tuple(out_names),
                lowering_input_output_aliases=(),
                sim_require_finite=True,
                sim_require_nnan=True,
                nc=nc,
            )
            return tuple(outs)

        devices = jax.devices()[:n_cores]
        mesh = Mesh(_np.asarray(devices), ("core",))
        n_outs = len(out_names)
        # No donation: the kernel writes every element of every output, so
        # the zero "output seed" operands can live on device and be reused
        # across timed calls.
        self.jitted = jax.jit(
            shard_map(
                _body,
                mesh=mesh,
                in_specs=(PartitionSpec("core"),) * (self.n_params + n_outs),
                out_specs=(PartitionSpec("core"),) * n_outs,
                check_rep=False,
            ),
            keep_unused=True,
        )
        from jax.sharding import NamedSharding

        self.sharding = NamedSharding(mesh, PartitionSpec("core"))
        self.dev_zeros = [
            jax.device_put(
                _np.zeros((n_cores * a.shape[0], *a.shape[1:]), a.dtype),
                self.sharding,
            )
            for a in out_avals
        ]

    def prep_args(self, in_maps):
        import jax
        import numpy as _np

        if self.dbg_name is not None:
            # 8-byte PA as uint32[1,2]; zero -> debugger store+halt skipped
            dbg = _np.zeros((1, 2), _np.uint32)
            in_maps = [{**m, self.dbg_name: dbg} for m in in_maps]
        return [
            jax.device_put(
                _np.concatenate(
                    [_np.asarray(m[name]) for m in in_maps], axis=0
                ),
                self.sharding,
            )
            for name in self.in_names
        ]

    def run(self, concat_in):
        import jax

        out = self.jitted(*concat_in, *self.dev_zeros)
        return jax.block_until_ready(out)

    def results(self, out_arrs):
        import numpy as _np

        return [
            {
                name: _np.asarray(out_arrs[i]).reshape(
                    self.n_cores, *self.out_avals[i].shape
                )[c]
                for i, name in enumerate(self.out_names)
            }
            for c in range(self.n_cores)
        ]


_RUNNER_CACHE = {}


def _get_runner(reps=1, with_bias_v=True, with_bias_p=True):
    key = (T, C, H, HD, reps, with_bias_v, with_bias_p)
    if key not in _RUNNER_CACHE:
        if key not in _NC_CACHE:
            _NC_CACHE[key] = _build_nc(
                T, C, H, HD, reps=reps,
                with_bias_v=with_bias_v, with_bias_p=with_bias_p,
            )
        _RUNNER_CACHE[key] = _Runner(_NC_CACHE[key], N_CORES)
    return _RUNNER_CACHE[key]


def _make_in_maps(x, w_attn, b_attn, w_proj, b_proj):
    w_qk, w_v, wp, b_qk, b_v, bp, masks = _prep_shared(
        w_attn, b_attn, w_proj, b_proj
    )
    x = np.asarray(x, np.float32)
    in_maps = []
    for i in range(N_CORES):
        xT = np.ascontiguousarray(x[i].T).astype(BF16)
        in_maps.append({
            "xT": xT, "w_qk": w_qk, "w_v": w_v, "w_proj": wp,
            "b_qk": b_qk, "b_v": b_v, "b_proj": bp, "masks": masks,
        })
    return in_maps


def _bias_flags(b_attn, b_proj):
    bv = np.asarray(b_attn, np.float32).reshape(H, 3, HD)[:, 2, :]
    return bool(bv.any()), bool(np.asarray(b_proj, np.float32).any())


def kernel(x, w_attn, b_attn, w_proj, b_proj):
    wbv, wbp = _bias_flags(b_attn, b_proj)
    runner = _get_runner(with_bias_v=wbv, with_bias_p=wbp)
    concat_in = runner.prep_args(
        _make_in_maps(x, w_attn, b_attn, w_proj, b_proj)
    )
    res = runner.results(runner.run(concat_in))
    return np.stack([res[i]["out"] for i in range(N_CORES)]).astype(np.float32)


def measure(x, w_attn, b_attn, w_proj, b_proj, iters=5, reps=1):
    """Warm wall-clock times (s) of the sharded on-device execution.

    reps > 1 uses a kernel variant whose compute body runs `reps` times
    per dispatch (device-side loop), so per-iteration HW time can be
    resolved despite the ~90 ms axon round-trip overhead."""
    import time

    wbv, wbp = _bias_flags(b_attn, b_proj)
    runner = _get_runner(reps=reps, with_bias_v=wbv, with_bias_p=wbp)
    concat_in = runner.prep_args(
        _make_in_maps(x, w_attn, b_attn, w_proj, b_proj)
    )
    runner.run(concat_in)  # warm-up / compile
    times = []
    for _ in range(iters):
        t0 = time.perf_counter()
        runner.run(concat_in)
        times.append(time.perf_counter() - t0)
    return times


# revision 58
# speedup vs baseline: 1.0478x; 1.0371x over previous
"""Causal self-attention on 8 TRN2 NeuronCores.

Sharding: pure data-parallel on batch (B=8 -> one batch element per core,
no collectives). Each core computes its full [T, C] output slice.

Per-core dataflow (all matmuls bf16 with fp32 PSUM accumulation):
  xT [C,T] (host-pretransposed) --+--> qkT = (w_qk^T @ xT) + b_qk  [2C, T]
                                  +--> V   = (xT^T @ w_v) + b_v    [T, C] (padded
                                       with a ones column per head: [.., 65])
  per head-pair u, per q-chunk (512):
    S^T[k,q] = k_h . q_h           (lhsT = kT_h slice, rhs = qT_h slice; the
                                    A/B head halves run concurrently in
                                    disjoint PE row groups)
    E = exp(S^T)                   (ACT, PSUM->SBUF, bf16 out; q pre-scaled
                                    by 1/8 on host so no separate scale op)
    E *= causal mask               (single strided-AP multiply covering only
                                    the 128-col diagonal blocks, on GpSimd)
    O'[d,q], s[q] = [v_h | 1]^T @ E  (augmented-ones row gives softmax sums;
                                      A and B go to slots of one PSUM tile)
    r = 1/s (DVE reciprocal_approx_fast, f32)
    R = partition_broadcast(r)     (GpSimd, SBUF->SBUF)
    Y^T block = O' * R             (DVE)
  Z = (Y^T)^T @ w_proj + b_proj    -> DMA PSUM -> DRAM out

Scheduling: the qkT GEMM is split per head-pair unit and its chains are
interleaved into the attention pipeline as PE filler, so the tensor engine
streams continuously (keeps the HAM clock-gate at 8/8) while exp/recip/
normalize latencies hide underneath. Input DMAs are split so the first
matmul can start ~2us in. Biases enter as K=1 rank-1 accumulate matmuls.
"""

import os
import sys
from contextlib import ExitStack

import numpy as np

try:
    import ml_dtypes
except ImportError:  # pragma: no cover
    sys.path.insert(0, "/opt/trn_rl_repo")
    import ml_dtypes

BF16 = ml_dtypes.bfloat16

B, T, C = 8, 1024, 1024
H, HD = 16, 64
N_CORES = 8

# Toggled by test harness to capture a hardware profile.
TRACE = False
LAST_EXEC_NS = None
LAST_RESULTS = None

_NC_CACHE = {}


def _build_nc(Tp, Cp, Hp, HDp, reps=1, with_bias_v=True, with_bias_p=True):
    import concourse.bass as bass
    import concourse.tile as tile
    from concourse import bacc, mybir

    bf = mybir.dt.bfloat16
    f32 = mybir.dt.float32
    AF = mybir.ActivationFunctionType

    P = 128
    CT = Cp // P            # c-tiles (contraction tiles)
    TT = Tp // P            # t-tiles
    QC = min(512, Tp)       # q-chunk width (free dim per matmul)
    NQ = Tp // QC           # q-chunks
    TCH = min(512, Tp)      # t-chunk width for qkT rhs
    TJ = Tp // TCH
    DIAG = QC // P          # diagonal k-tiles per q-chunk
    M2C = 2 * Cp // P       # qk m-chunks
    VJ = Cp // QC           # v/proj column chunks
    NU = Hp // 2            # head-pair units
    NIT = NU * NQ           # attention iterations (unit, q-chunk)

    nc = bacc.Bacc("TRN2", target_bir_lowering=False, debug=False)

    xT_d = nc.declare_dram_parameter("xT", [Cp, Tp], bf, isOutput=False)
    wqk_d = nc.declare_dram_parameter("w_qk", [Cp, 2 * Cp], bf, isOutput=False)
    wv_d = nc.declare_dram_parameter("w_v", [Cp, Cp], bf, isOutput=False)
    wp_d = nc.declare_dram_parameter("w_proj", [Cp, Cp], bf, isOutput=False)
    bqk_d = nc.declare_dram_parameter("b_qk", [M2C, P], f32, isOutput=False)
    bv_d = nc.declare_dram_parameter("b_v", [1, Cp], bf, isOutput=False)
    bp_d = nc.declare_dram_parameter("b_proj", [1, Cp], bf, isOutput=False)
    # lower-tri [k<=q] 128x128 block, replicated DIAG times along the middle
    # axis to match the strided diagonal-block AP of the E tiles
    mask_d = nc.declare_dram_parameter("masks", [P, DIAG, P], bf, isOutput=False)
    out_d = nc.declare_dram_parameter("out", [Tp, Cp], f32, isOutput=True)

    with tile.TileContext(nc) as tc, ExitStack() as ctx:
        consts = ctx.enter_context(tc.tile_pool(name="consts", bufs=1))
        # E tiles are flat [P, cols]; qj=0 needs 2560 cols, qj=1 needs 4608
        epool0 = ctx.enter_context(tc.tile_pool(name="epool0", bufs=2))
        epool1 = ctx.enter_context(tc.tile_pool(name="epool1", bufs=2))
        rpool2 = ctx.enter_context(tc.tile_pool(name="rpool2", bufs=2))
        zpool = ctx.enter_context(tc.tile_pool(name="zpool", bufs=2))
        # qkT head-pair tiles rotate through 4 slots (live qk-emit -> S use)
        qkpool = ctx.enter_context(tc.tile_pool(name="qkpool", bufs=4))
        # S-pair tiles and the combined AV accumulator share one 3-slot pool
        # (each [P, 2, QC] f32 = 2 banks -> 6 banks) + 2 banks for GEMM chains
        psatt = ctx.enter_context(tc.tile_pool(name="psatt", bufs=3, space="PSUM"))
        psmm = ctx.enter_context(tc.tile_pool(name="psmm", bufs=2, space="PSUM"))

        # ---- persistent SBUF buffers ----
        xT = consts.tile([P, CT, Tp], bf)
        wqk = consts.tile([P, CT, 2 * Cp], bf)
        wv = consts.tile([P, CT, Cp], bf)
        wp = consts.tile([P, CT, Cp], bf)
        # per head: [ones | 31 pad | v0..v63] (96 cols) -> softmax-sum row
        # lands at PSUM partition 0 (where the custom-DVE reciprocal reads)
        # and O' at partitions 32:96 (read as 2x32-partition slices); 96-col
        # weights keep the AV LDWEIGHTS cheaper than a full 128-col load
        VW = 32 + HDp
        Vp = consts.tile([P, TT, Hp, VW], bf)
        YT = consts.tile([P, CT, Tp], bf)
        bqk = consts.tile([P, M2C], f32)
        bv = bp = None
        if with_bias_v:
            bv = consts.tile([1, Cp], bf)
        if with_bias_p:
            bp = consts.tile([1, Cp], bf)
        ones = None
        if with_bias_v or with_bias_p:
            ones = consts.tile([1, max(QC, P)], bf)
        masks = consts.tile([P, DIAG, P], bf)

        # ---- input DMAs, priority-ordered and split for early PE start ----
        # wqk column pairs: unit u needs cols [128u,128u+128) and Cp+[...]
        def _dma_wqk(lo_u, hi_u):
            w = (hi_u - lo_u) * P
            for base in (0, Cp):
                nc.sync.dma_start(
                    wqk[:, :, base + lo_u * P: base + lo_u * P + w],
                    wqk_d[:, base + lo_u * P: base + lo_u * P + w].rearrange(
                        "(ct p) n -> p ct n", p=P
                    ),
                )

        # Each dma_start costs ~1.1us of serial issue on the sync engine, so
        # the first qk chain's dependencies (bqk, wqk unit 0, xT) are issued
        # first and with minimal instruction count (xT in 2-ct chunks).
        nc.sync.dma_start(bqk[:], bqk_d.rearrange("m p -> p m"))
        _dma_wqk(0, 1)  # unit 0 only
        for ct in range(0, CT, 2):
            nc.sync.dma_start(
                xT[:, ct:ct + 2, :],
                xT_d[ct * P: (ct + 2) * P, :].rearrange(
                    "(a p) t -> p a t", p=P
                ),
            )
        _dma_wqk(1, 2)
        nc.sync.dma_start(wv[:], wv_d.rearrange("(ct p) n -> p ct n", p=P))
        nc.sync.dma_start(masks[:], mask_d[:])
        _dma_wqk(2, NU)  # remaining units
        nc.sync.dma_start(wp[:], wp_d.rearrange("(ct p) n -> p ct n", p=P))
        if with_bias_v:
            nc.sync.dma_start(bv[:], bv_d[:])
        if with_bias_p:
            nc.sync.dma_start(bp[:], bp_d[:])
        if ones is not None:
            nc.gpsimd.memset(ones[:], 1.0)
        nc.gpsimd.memset(Vp[:], 1.0)  # ones column survives; rest overwritten

        qkt_tiles = {}

        def _emit_qk_chain(u, half, tj):
            # one qkT chain: [P, TCH] psum accumulated over CT, then bias+copy
            # into slot `half` (0=q, 1=k) of unit u's rotating qkT tile
            if u not in qkt_tiles:
                qkt_tiles[u] = qkpool.tile(
                    [P, 2, Tp], bf, tag="qkt", name=f"qkt_{u}"
                )
            qt = qkt_tiles[u]
            m = u if half == 0 else M2C // 2 + u
            msl = slice(m * P, (m + 1) * P)
            tsl = slice(tj * TCH, (tj + 1) * TCH)
            ps = psmm.tile([P, TCH], f32, tag="mm")
            for ct in range(CT):
                nc.tensor.matmul(
                    ps[:], lhsT=wqk[:, ct, msl], rhs=xT[:, ct, tsl],
                    start=(ct == 0), stop=(ct == CT - 1),
                )
            nc.vector.tensor_scalar_add(qt[:, half, tsl], ps[:], bqk[:, m:m + 1])

        def _emit_qk_unit(u):
            for half in (0, 1):
                for tj in range(TJ):
                    _emit_qk_chain(u, half, tj)

        def _emit_v_chunk(ti, vj):
            tsl = slice(ti * P, (ti + 1) * P)
            vsl = slice(vj * QC, (vj + 1) * QC)
            ps = psmm.tile([P, QC], f32, tag="mm")
            for ct in range(CT):
                nc.tensor.matmul(
                    ps[:], lhsT=xT[:, ct, tsl], rhs=wv[:, ct, vsl],
                    start=(ct == 0),
                    stop=(ct == CT - 1 and not with_bias_v),
                )
            if with_bias_v:
                nc.tensor.matmul(
                    ps[:], lhsT=ones[0:1, 0:P], rhs=bv[0:1, vsl],
                    start=False, stop=True,
                )
            hpc = QC // HDp  # heads per chunk
            nc.vector.tensor_copy(
                out=Vp[:, ti, vj * hpc:(vj + 1) * hpc, 32:VW],
                in_=ps[:].rearrange("p (h d) -> p h d", d=HDp),
            )

        # ---- attention pipeline pieces ----
        # Head pairs: head 2u on partitions 0:64, head 2u+1 on 64:128 of
        # qkT chunk u (q) / M2C//2+u (k). The A/B matmuls use disjoint PE
        # row groups (tile_position auto-derived from base_partition), so
        # they run concurrently in the array.
        E_tiles = {}

        def _ecols(qj):
            nk = DIAG * (qj + 1)
            return nk * QC

        def _emit_S(it):
            u, qj = divmod(it, NQ)
            nk = DIAG * (qj + 1)
            q0 = qj * QC
            pool = epool0 if qj == 0 else epool1
            E_A = pool.tile([P, _ecols(qj)], bf, tag=f"E{qj}")
            E_B = pool.tile([P, _ecols(qj)], bf, tag=f"E{qj}")
            E_tiles[it] = (E_A, E_B)
            qt = qkt_tiles[u]
            qk_parts = (
                (qt[0:HDp, 0, :], qt[0:HDp, 1, :]),
                (qt[HDp:P, 0, :], qt[HDp:P, 1, :]),
            )
            for g in range(nk // 2):
                offg = max(0, P * (2 * g - DIAG * qj))
                ps_h = [
                    psatt.tile([P, 2, QC], f32, tag="satt",
                               name=f"ps_s_{u}_{qj}_{g}_{hh}")
                    for hh in range(2)
                ]
                for r2 in range(2):
                    ki = 2 * g + r2
                    ksl = slice(ki * P, (ki + 1) * P)
                    for half, (qT, kT) in enumerate(qk_parts):
                        nc.tensor.matmul(
                            ps_h[half][:, r2, offg:],
                            lhsT=kT[:, ksl],
                            rhs=qT[:, q0 + offg:q0 + QC],
                            start=True, stop=True,
                        )
                for half, E in enumerate((E_A, E_B)):
                    eap = E[:, 2 * g * QC: (2 * g + 2) * QC].rearrange(
                        "p (a q) -> p a q", a=2
                    )[:, :, offg:]
                    nc.scalar.activation(eap, ps_h[half][:, :, offg:], AF.Exp)
            # causal mask: strided-AP multiplies over the DIAG diagonal
            # 128-col blocks (stride QC+P), on DVE (no cross-engine waits
            # beyond exp, so the DVE FIFO never blocks on GpSimd).
            # Split 3+1 so the rearrange slice stays inside the E tile.
            dbase = DIAG * qj * QC
            for E in (E_A, E_B):
                dap = E[:, dbase: dbase + (DIAG - 1) * (QC + P)].rearrange(
                    "p (r j) -> p r j", r=DIAG - 1
                )[:, :, 0:P]
                nc.vector.tensor_mul(out=dap, in0=dap, in1=masks[:, 0:DIAG - 1, :])
                last = dbase + (DIAG - 1) * (QC + P)
                nc.vector.tensor_mul(
                    out=E[:, last: last + P], in0=E[:, last: last + P],
                    in1=masks[:, DIAG - 1, :],
                )

        tails = {}

        def _emit_tail_av(it):
            u, qj = divmod(it, NQ)
            nk = DIAG * (qj + 1)
            E_A, E_B = E_tiles.pop(it)
            # O'[d,q] at partitions 64:128 + sums row at partition 0, via the
            # [ones|pad|v] columns of Vp; A and B into the two bank-slots of
            # one PSUM tile
            pav = psatt.tile([P, 2, QC], f32, tag="satt",
                             name=f"pav_{u}_{qj}")
            for g, E, h in ((0, E_A, 2 * u), (1, E_B, 2 * u + 1)):
                for ki in range(nk):
                    off = max(0, P * (ki - DIAG * qj))
                    nc.tensor.matmul(
                        pav[0:VW, g, off:],
                        lhsT=Vp[:, ki, h, :],
                        rhs=E[:, ki * QC + off: (ki + 1) * QC],
                        start=(ki == 0), stop=(ki == nk - 1),
                    )
            rrow = rpool2.tile([1, 2, QC], f32, tag="rrow")
            rb = rpool2.tile([HDp, 2, QC], f32, tag="rb")
            nc.vector.reciprocal_approx_fast(rrow[:], pav[0:1, :, :])
            # GpSimd runs ONLY partition_broadcast, so its ucode library
            # never swaps (each swap costs ~6us of hidden latency)
            nc.gpsimd.partition_broadcast(rb[:], rrow[:])
            tails[it] = (pav, rb)

        def _emit_tail_mul(it):
            # normalize multiplies, deferred one iteration so this DVE work
            # never waits on the GpSimd broadcast at the FIFO head
            u, qj = divmod(it, NQ)
            pav, rb = tails.pop(it)
            qsl = slice(qj * QC, qj * QC + QC)
            # O' sits at pav partitions 32:96; a 64-partition engine access
            # may only start at 0/64, so normalize in 32-partition halves
            for g, y0 in ((0, 0), (1, HDp)):
                nc.vector.tensor_mul(
                    out=YT[y0:y0 + 32, u, qsl],
                    in0=pav[32:64, g, :], in1=rb[0:32, g, :],
                )
                nc.vector.tensor_mul(
                    out=YT[y0 + 32:y0 + HDp, u, qsl],
                    in0=pav[64:96, g, :], in1=rb[32:HDp, g, :],
                )

        def _emit_proj_chunk(ti, zj):
            tsl = slice(ti * P, (ti + 1) * P)
            zsl = slice(zj * QC, (zj + 1) * QC)
            ps = psmm.tile([P, QC], f32, tag="mm")
            for ct in range(CT):
                nc.tensor.matmul(
                    ps[:], lhsT=YT[:, ct, tsl], rhs=wp[:, ct, zsl],
                    start=(ct == 0),
                    stop=(ct == CT - 1 and not with_bias_p),
                )
            if with_bias_p:
                nc.tensor.matmul(
                    ps[:], lhsT=ones[0:1, 0:P], rhs=bp[0:1, zsl],
                    start=False, stop=True,
                )
            zt = zpool.tile([P, QC], f32, tag="zt")
            nc.vector.tensor_copy(out=zt[:], in_=ps[:])
            nc.sync.dma_start(out_d[tsl, zsl], zt[:])

        def _emit_body():
            qkt_tiles.clear()
            # prologue: qkT for units 0-1, S(0), V interleaved with S(1)
            _emit_qk_unit(0)
            _emit_qk_unit(1)
            _emit_S(0)
            for ti in range(TT // 2):
                for vj in range(VJ):
                    _emit_v_chunk(ti, vj)
            _emit_S(1)
            for ti in range(TT // 2, TT):
                for vj in range(VJ):
                    _emit_v_chunk(ti, vj)
            _emit_tail_av(0)
            # steady state: S one iteration ahead of the AV tail, normalize
            # multiplies one further behind, two qkT chains per iteration as
            # PE filler. Unit u's chains finish emitting before S(2u).
            fill = []
            for u in range(2, NU):
                for half in (0, 1):
                    for tj in range(TJ):
                        fill.append((u, half, tj))
            nfill = len(fill)
            fi = 0
            _emit_tail_mul(0)
            for it in range(2, NIT):
                _emit_S(it)
                _emit_tail_av(it - 1)
                _emit_tail_mul(it - 1)
                for _ in range(min(2, nfill - fi)):
                    _emit_qk_chain(*fill[fi])
                    fi += 1
            while fi < nfill:
                _emit_qk_chain(*fill[fi])
                fi += 1
            _emit_tail_av(NIT - 1)
            _emit_tail_mul(NIT - 1)
            for ti in range(TT):
                for zj in range(VJ):
                    _emit_proj_chunk(ti, zj)

        if reps == 1:
            _emit_body()
        else:
            hint = (
                mybir.EngineType.PE,
                mybir.EngineType.DVE,
                mybir.EngineType.Activation,
            )
            with tc.For_i(0, reps, 1, hint_engines=hint):
                _emit_body()

    nc.finalize()
    return nc


def _prep_shared(w_attn, b_attn, w_proj, b_proj):
    """Host-side layout marshalling of the replicated weights (bf16 cast,
    per-head q/k/v column gather, exact 1/8 q pre-scale)."""
    wr = np.asarray(w_attn, np.float32).reshape(C, H, 3, HD)
    w_q = (wr[:, :, 0, :] * np.float32(0.125)).reshape(C, C)
    w_k = wr[:, :, 1, :].reshape(C, C)
    w_qk = np.ascontiguousarray(
        np.concatenate([w_q, w_k], axis=1)
    ).astype(BF16)
    w_v = np.ascontiguousarray(wr[:, :, 2, :].reshape(C, C)).astype(BF16)

    br = np.asarray(b_attn, np.float32).reshape(H, 3, HD)
    # per-partition column layout for the qkT copyback bias: [M2C, 128] f32
    b_qk = np.ascontiguousarray(
        np.concatenate(
            [(br[:, 0, :] * np.float32(0.125)).reshape(C), br[:, 1, :].reshape(C)]
        ).reshape(2 * C // 128, 128)
    )
    b_v = np.ascontiguousarray(br[:, 2, :].reshape(1, C)).astype(BF16)

    wp = np.ascontiguousarray(np.asarray(w_proj, np.float32)).astype(BF16)
    bp = np.ascontiguousarray(np.asarray(b_proj, np.float32).reshape(1, C)).astype(BF16)

    # lower-tri 128x128 (k <= q within a diagonal block), replicated DIAG
    # times to match the strided diagonal-block AP
    DIAGv = min(512, T) // 128
    k_idx = np.arange(128)[:, None]
    q_idx = np.arange(128)[None, :]
    tri = (k_idx <= q_idx)
    masks = np.ascontiguousarray(
        np.broadcast_to(tri[:, None, :], (128, DIAGv, 128))
    ).astype(BF16)
    return w_qk, w_v, wp, b_qk, b_v, bp, masks


class _Runner:
    """Cached jit(shard_map) executor for a prebuilt Bass module across
    N cores — same lowering as bass2jax.run_bass_via_pjrt, but reusable
    across calls so warm executions can be timed."""

    def __init__(self, nc, n_cores):
        import jax
        import numpy as _np
        from jax.sharding import Mesh, PartitionSpec
        try:
            from jax.experimental.shard_map import shard_map
        except ImportError:
            from jax.shard_map import shard_map
        from concourse import bass2jax, mybir

        bass2jax.install_neuronx_cc_hook()
        assert not nc.dbg_callbacks
        self.dbg_name = nc.dbg_addr.name if nc.dbg_addr is not None else None
        partition_name = (
            nc.partition_id_tensor.name if nc.partition_id_tensor else None
        )

        in_names, out_names, out_avals = [], [], []
        for alloc in nc.m.functions[0].allocations:
            if not isinstance(alloc, mybir.MemoryLocationSet):
                continue
            name = alloc.memorylocations[0].name
            if alloc.kind == "ExternalInput":
                if name != partition_name:
                    in_names.append(name)
            elif alloc.kind == "ExternalOutput":
                out_names.append(name)
                out_avals.append(
                    jax.core.ShapedArray(
                        tuple(alloc.tensor_shape), mybir.dt.np(alloc.dtype)
                    )
                )
        self.n_params = len(in_names)
        self.in_names = list(in_names)
        self.out_names = out_names
        self.out_avals = out_avals
        self.n_cores = n_cores
        all_names = in_names + out_names
        if partition_name is not None:
            all_names = all_names + [partition_name]

        def _body(*args):
            operands = list(args)
            if partition_name is not None:
                operands.append(bass2jax.partition_id_tensor())
            outs = bass2jax._bass_exec_p.bind(
                *operands,
                out_avals=tuple(out_avals),
                in_names=tuple(all_names),
                out_names=tuple(out_names),
                lowering_input_output_aliases=(),
                sim_require_finite=True,
                sim_require_nnan=True,
                nc=nc,
            )
            return tuple(outs)

        devices = jax.devices()[:n_cores]
        mesh = Mesh(_np.asarray(devices), ("core",))
        n_outs = len(out_names)
        # No donation: the kernel writes every element of every output, so
        # the zero "output seed" operands can live on device and be reused
        # across timed calls.
        self.jitted = jax.jit(
            shard_map(
                _body,
                mesh=mesh,
                in_specs=(PartitionSpec("core"),) * (self.n_params + n_outs),
                out_specs=(PartitionSpec("core"),) * n_outs,
                check_rep=False,
            ),
            keep_unused=True,
        )
        from jax.sharding import NamedSharding

        self.sharding = NamedSharding(mesh, PartitionSpec("core"))
        self.dev_zeros = [
            jax.device_put(
                _np.zeros((n_cores * a.shape[0], *a.shape[1:]), a.dtype),
                self.sharding,
            )
            for a in out_avals
        ]

    def prep_args(self, in_maps):
        import jax
        import numpy as _np

        if self.dbg_name is not None:
            # 8-byte PA as uint32[1,2]; zero -> debugger store+halt skipped
            dbg = _np.zeros((1, 2), _np.uint32)
            in_maps = [{**m, self.dbg_name: dbg} for m in in_maps]
        return [
            jax.device_put(
                _np.concatenate(
                    [_np.asarray(m[name]) for m in in_maps], axis=0
                ),
                self.sharding,
            )
            for name in self.in_names
        ]

    def run(self, concat_in):
        import jax

        out = self.jitted(*concat_in, *self.dev_zeros)
        return jax.block_until_ready(out)

    def results(self, out_arrs):
        import numpy as _np

        return [
            {
                name: _np.asarray(out_arrs[i]).reshape(
                    self.n_cores, *self.out_avals[i].shape
                )[c]
                for i, name in enumerate(self.out_names)
            }
            for c in range(self.n_cores)
        ]


_RUNNER_CACHE = {}


def _get_runner(reps=1, with_bias_v=True, with_bias_p=True):
    key = (T, C, H, HD, reps, with_bias_v, with_bias_p)
    if key not in _RUNNER_CACHE:
        if key not in _NC_CACHE:
            _NC_CACHE[key] = _build_nc(
                T, C, H, HD, reps=reps,
                with_bias_v=with_bias_v, with_bias_p=with_bias_p,
            )
        _RUNNER_CACHE[key] = _Runner(_NC_CACHE[key], N_CORES)
    return _RUNNER_CACHE[key]


def _make_in_maps(x, w_attn, b_attn, w_proj, b_proj):
    w_qk, w_v, wp, b_qk, b_v, bp, masks = _prep_shared(
        w_attn, b_attn, w_proj, b_proj
    )
    x = np.asarray(x, np.float32)
    in_maps = []
    for i in range(N_CORES):
        xT = np.ascontiguousarray(x[i].T).astype(BF16)
        in_maps.append({
            "xT": xT, "w_qk": w_qk, "w_v": w_v, "w_proj": wp,
            "b_qk": b_qk, "b_v": b_v, "b_proj": bp, "masks": masks,
        })
    return in_maps


def _bias_flags(b_attn, b_proj):
    bv = np.asarray(b_attn, np.float32).reshape(H, 3, HD)[:, 2, :]
    return bool(bv.any()), bool(np.asarray(b_proj, np.float32).any())


def kernel(x, w_attn, b_attn, w_proj, b_proj):
    wbv, wbp = _bias_flags(b_attn, b_proj)
    runner = _get_runner(with_bias_v=wbv, with_bias_p=wbp)
    concat_in = runner.prep_args(
        _make_in_maps(x, w_attn, b_attn, w_proj, b_proj)
    )
    res = runner.results(runner.run(concat_in))
    return np.stack([res[i]["out"] for i in range(N_CORES)]).astype(np.float32)


def measure(x, w_attn, b_attn, w_proj, b_proj, iters=5, reps=1):
    """Warm wall-clock times (s) of the sharded on-device execution.

    reps > 1 uses a kernel variant whose compute body runs `reps` times
    per dispatch (device-side loop), so per-iteration HW time can be
    resolved despite the ~90 ms axon round-trip overhead."""
    import time

    wbv, wbp = _bias_flags(b_attn, b_proj)
    runner = _get_runner(reps=reps, with_bias_v=wbv, with_bias_p=wbp)
    concat_in = runner.prep_args(
        _make_in_maps(x, w_attn, b_attn, w_proj, b_proj)
    )
    runner.run(concat_in)  # warm-up / compile
    times = []
    for _ in range(iters):
        t0 = time.perf_counter()
        runner.run(concat_in)
        times.append(time.perf_counter() - t0)
    return times
